# revision 3
# baseline (speedup 1.0000x reference)
"""Grouped linear (MoE routing) kernel for 8 Trainium2 NeuronCores, v3.

out[n] = x[n] @ weight[g[n]].T + bias[g[n]]

Expert-parallel (core g owns group g, rows padded to common capacity
C), with bf16 matmul operands (PSUM accumulation and bias add stay
fp32). bf16 runs at the same 1 col/cycle PE rate as fp32r but halves
DMA bytes and SBUF footprint, so BOTH the x shard and all of W^T are
SBUF-resident and the kernel is a single n-outer sweep:

  for n in 0..3:                 # 512-col output slices
    for m in 0..C/128-1:
      psum = sum_ko xT[m,ko] @ w[n,ko]   (16 matmuls, fp32 acc)
      out[m,n] = psum + bias[n]          (VectorE, fp32)

Startup: the PE begins after ~2 us — the first x tile's leading ko
chunk plus the first W slice. While the n=0 W slab streams in 128 KB
ko-slices, the m=0 group tracks the DMA at the p-state ramp rate, so
almost nothing is lost. Bias comes in per-n 256 KB chunks ordered so
the first eviction never waits. The out-staging pool is deep enough
(one sweep) that evict DMAs queued behind the prefetch stream never
back-pressure the PE.
"""

import math
import sys

for _p in ("/opt/trn_rl_repo", "/root/.axon_site/_ro/trn_rl_repo"):
    if _p not in sys.path:
        sys.path.append(_p)

import numpy as np
import ml_dtypes

from concourse import bacc, mybir, tile
from concourse.bass_utils import run_bass_kernel_spmd

P = 128
D_IN = 2048
D_OUT = 2048
KO = D_IN // P  # 16 k-subtiles
N_TILE = 512
N_TILES = D_OUT // N_TILE  # 4
NUM_GROUPS = 8
N_CORES = 8

_nc_cache: dict = {}


def build_program(C: int, repeat: int = 1):
    """Per-core Bass program for row capacity C, bf16 operands.

    repeat > 1 re-runs the compute sweep over the resident data; used
    for steady-state repeat-differencing timing.
    """
    key = (C, repeat)
    if key in _nc_cache:
        return _nc_cache[key]
    assert C % P == 0
    m_tiles = C // P
    f32 = mybir.dt.float32
    bf16 = mybir.dt.bfloat16

    nc = bacc.Bacc(
        "TRN2", target_bir_lowering=False, debug=False, num_devices=N_CORES
    )
    # Blocked HBM layouts (prepared host-side):
    #   xT[m, kp, ko, j] = x[m*128+j, ko*128+kp]     (bf16, 4 KB runs)
    #   wT[n, kp, ko, nn] = W^T[ko*128+kp, n*512+nn] (bf16, 1 KB runs)
    xT = nc.dram_tensor("xT", [m_tiles, P, KO, P], bf16, kind="ExternalInput").ap()
    wT = nc.dram_tensor(
        "wT", [N_TILES, P, KO, N_TILE], bf16, kind="ExternalInput"
    ).ap()
    bb = nc.dram_tensor("bb", [P, D_OUT], f32, kind="ExternalInput").ap()
    out = nc.dram_tensor("out", [C, D_OUT], f32, kind="ExternalOutput").ap()

    with tile.TileContext(nc) as tc:
        with (
            tc.tile_pool(name="xpool", bufs=1) as xpool,
            tc.tile_pool(name="wpool", bufs=1) as wpool,
            tc.tile_pool(name="cpool", bufs=1) as cpool,
            tc.tile_pool(name="opool", bufs=max(18, m_tiles + 1)) as opool,
            tc.tile_pool(name="pspool", bufs=8, space="PSUM") as pspool,
        ):
            x_sb = xpool.tile([P, m_tiles, KO, P], bf16)
            w_sb = wpool.tile([P, N_TILES, KO, N_TILE], bf16)
            b_sb = [
                cpool.tile([P, N_TILE], f32, name=f"b_sb{_n}")
                for _n in range(N_TILES)
            ]

            def x_tile_dma(m):
                nc.sync.dma_start(x_sb[:, m], xT[m])

            def w_slice_dma(n, ko0, ko1):
                nc.sync.dma_start(w_sb[:, n, ko0:ko1], wT[n, :, ko0:ko1])

            # DMA issue order == HBM service order (single serialized DMA
            # resource). x0 and x1 land first (the only PE-idle stretch),
            # then the n=0 W slab streams uninterrupted while a ko-outer
            # m={0,1} pair consumes each slice twice — 426 ns of PE work
            # per 364 ns slice, so the PE tracks the slab PE-bound.
            # 2-ko W slices for the n=0 slab: 728 ns transfer each, just
            # above the ~625 ns serialized HWDGE setup, so the slab
            # cadence is transfer-bound. Later slabs are fully prefetched
            # anyway and go as single 2 MB DMAs.
            x_tile_dma(0)
            if m_tiles > 1:
                x_tile_dma(1)
            for ko in range(0, KO, 2):
                w_slice_dma(0, ko, ko + 2)
            for m in range(2, min(4, m_tiles)):
                x_tile_dma(m)
            nc.sync.dma_start(b_sb[0][:], bb[:, 0:N_TILE])
            for m in range(4, m_tiles):
                x_tile_dma(m)
            for n in range(1, N_TILES):
                w_slice_dma(n, 0, KO)
                ns = slice(n * N_TILE, (n + 1) * N_TILE)
                nc.sync.dma_start(b_sb[n][:], bb[:, ns])

            def do_group(m, n, split_evict=False):
                ps = pspool.tile([P, N_TILE], f32, tag="ps")
                for ko in range(KO):
                    nc.tensor.matmul(
                        ps,
                        x_sb[:, m, ko],
                        w_sb[:, n, ko],
                        start=(ko == 0),
                        stop=(ko == KO - 1),
                    )
                ms = slice(m * P, (m + 1) * P)
                o_sb = opool.tile([P, N_TILE], f32, tag="o")
                if split_evict:
                    # last group: halve the post-matmul critical path
                    for h in range(2):
                        hs = slice(h * (N_TILE // 2), (h + 1) * (N_TILE // 2))
                        ns = slice(
                            n * N_TILE + h * (N_TILE // 2),
                            n * N_TILE + (h + 1) * (N_TILE // 2),
                        )
                        nc.vector.tensor_add(o_sb[:, hs], ps[:, hs], b_sb[n][:, hs])
                        nc.sync.dma_start(out[ms, ns], o_sb[:, hs])
                else:
                    ns = slice(n * N_TILE, (n + 1) * N_TILE)
                    nc.vector.tensor_add(o_sb[:], ps, b_sb[n][:])
                    nc.sync.dma_start(out[ms, ns], o_sb[:])

            def evict(ps, m, n):
                ms = slice(m * P, (m + 1) * P)
                ns = slice(n * N_TILE, (n + 1) * N_TILE)
                o_sb = opool.tile([P, N_TILE], f32, tag="o")
                nc.vector.tensor_add(o_sb[:], ps, b_sb[n][:])
                nc.sync.dma_start(out[ms, ns], o_sb[:])

            def start_pair(n):
                # ko-outer over m={0,1}: both matmuls of each ko need only
                # W slice ko, so each arriving slice feeds 2 matmuls.
                ps0 = pspool.tile([P, N_TILE], f32, tag="ps", name="ps0")
                ps1 = pspool.tile([P, N_TILE], f32, tag="ps", name="ps1")
                for ko in range(KO):
                    for m, ps in ((0, ps0), (1, ps1)):
                        nc.tensor.matmul(
                            ps,
                            x_sb[:, m, ko],
                            w_sb[:, n, ko],
                            start=(ko == 0),
                            stop=(ko == KO - 1),
                        )
                evict(ps0, 0, n)
                evict(ps1, 1, n)

            for _rep in range(repeat):
                for n in range(N_TILES):
                    m0 = 0
                    if _rep == 0 and n == 0 and m_tiles >= 2:
                        start_pair(n)
                        m0 = 2
                    for m in range(m0, m_tiles):
                        last = (
                            _rep == repeat - 1
                            and n == N_TILES - 1
                            and m == m_tiles - 1
                        )
                        do_group(m, n, split_evict=last)

    nc.compile()
    _nc_cache[key] = nc
    return nc


def shard_inputs(x, weight, bias, group_indices):
    """Host-side expert-parallel sharding. Returns (in_maps, perm,
    offsets, counts, C)."""
    n_rows = x.shape[0]
    gi = np.asarray(group_indices)
    perm = np.argsort(gi, kind="stable")
    counts = np.bincount(gi, minlength=NUM_GROUPS).astype(np.int64)
    offsets = np.zeros(NUM_GROUPS + 1, dtype=np.int64)
    np.cumsum(counts, out=offsets[1:])
    C = max(P, int(math.ceil(counts.max() / P)) * P)
    assert C // P <= 34, "x shard no longer fits SBUF-resident"

    x_sorted = x[perm] if not np.array_equal(perm, np.arange(n_rows)) else x
    m_tiles = C // P
    xbf = x_sorted.astype(ml_dtypes.bfloat16)
    wbf = np.asarray(weight).astype(ml_dtypes.bfloat16)
    in_maps = []
    for g in range(NUM_GROUPS):
        ng = int(counts[g])
        xg = np.zeros((C, D_IN), dtype=ml_dtypes.bfloat16)
        xg[:ng] = xbf[offsets[g] : offsets[g] + ng]
        # blocked layouts — see build_program
        xb = np.ascontiguousarray(
            xg.reshape(m_tiles, P, KO, P).transpose(0, 3, 2, 1)
        )
        wb = np.ascontiguousarray(
            wbf[g].T.reshape(KO, P, N_TILES, N_TILE).transpose(2, 1, 0, 3)
        )
        in_maps.append(
            {
                "xT": xb,
                "wT": wb,
                "bb": np.ascontiguousarray(
                    np.broadcast_to(bias[g].astype(np.float32), (P, D_OUT))
                ),
            }
        )
    return in_maps, perm, offsets, counts, C


def unshard_output(results, perm, offsets, counts, n_rows):
    out = np.empty((n_rows, D_OUT), dtype=np.float32)
    for g in range(NUM_GROUPS):
        ng = int(counts[g])
        out[perm[offsets[g] : offsets[g] + ng]] = results[g]["out"][:ng]
    return out


def kernel(x, weight, bias, group_indices):
    x = np.asarray(x, dtype=np.float32)
    weight = np.asarray(weight, dtype=np.float32)
    bias = np.asarray(bias, dtype=np.float32)
    group_indices = np.asarray(group_indices)
    assert x.shape[1] == D_IN and weight.shape == (NUM_GROUPS, D_OUT, D_IN)

    in_maps, perm, offsets, counts, C = shard_inputs(
        x, weight, bias, group_indices
    )
    nc = build_program(C)
    res = run_bass_kernel_spmd(nc, in_maps, core_ids=list(range(N_CORES)))
    return unshard_output(res.results, perm, offsets, counts, x.shape[0])


# revision 4
# speedup vs baseline: 1.0109x; 1.0109x over previous
"""Grouped linear (MoE routing) kernel for 8 Trainium2 NeuronCores, v3.

out[n] = x[n] @ weight[g[n]].T + bias[g[n]]

Expert-parallel (core g owns group g, rows padded to common capacity
C), with bf16 matmul operands (PSUM accumulation and bias add stay
fp32). bf16 runs at the same 1 col/cycle PE rate as fp32r but halves
DMA bytes and SBUF footprint, so BOTH the x shard and all of W^T are
SBUF-resident and the kernel is a single n-outer sweep:

  for n in 0..3:                 # 512-col output slices
    for m in 0..C/128-1:
      psum = sum_ko xT[m,ko] @ w[n,ko]   (16 matmuls, fp32 acc)
      out[m,n] = psum + bias[n]          (VectorE, fp32)

Startup: the PE begins after ~2 us — the first x tile's leading ko
chunk plus the first W slice. While the n=0 W slab streams in 128 KB
ko-slices, the m=0 group tracks the DMA at the p-state ramp rate, so
almost nothing is lost. Bias comes in per-n 256 KB chunks ordered so
the first eviction never waits. The out-staging pool is deep enough
(one sweep) that evict DMAs queued behind the prefetch stream never
back-pressure the PE.
"""

import math
import sys

for _p in ("/opt/trn_rl_repo", "/root/.axon_site/_ro/trn_rl_repo"):
    if _p not in sys.path:
        sys.path.append(_p)

import numpy as np
import ml_dtypes

from concourse import bacc, mybir, tile
from concourse.bass_utils import run_bass_kernel_spmd

P = 128
D_IN = 2048
D_OUT = 2048
KO = D_IN // P  # 16 k-subtiles
N_TILE = 512
N_TILES = D_OUT // N_TILE  # 4
NUM_GROUPS = 8
N_CORES = 8

_nc_cache: dict = {}


def build_program(C: int, repeat: int = 1):
    """Per-core Bass program for row capacity C, bf16 operands.

    repeat > 1 re-runs the compute sweep over the resident data; used
    for steady-state repeat-differencing timing.
    """
    key = (C, repeat)
    if key in _nc_cache:
        return _nc_cache[key]
    assert C % P == 0
    m_tiles = C // P
    f32 = mybir.dt.float32
    bf16 = mybir.dt.bfloat16

    nc = bacc.Bacc(
        "TRN2", target_bir_lowering=False, debug=False, num_devices=N_CORES
    )
    # Blocked HBM layouts (prepared host-side):
    #   xT[m, kp, ko, j] = x[m*128+j, ko*128+kp]     (bf16, 4 KB runs)
    #   wT[n, kp, ko, nn] = W^T[ko*128+kp, n*512+nn] (bf16, 1 KB runs)
    xT = nc.dram_tensor("xT", [m_tiles, P, KO, P], bf16, kind="ExternalInput").ap()
    wT = nc.dram_tensor(
        "wT", [N_TILES, P, KO, N_TILE], bf16, kind="ExternalInput"
    ).ap()
    bb = nc.dram_tensor("bb", [P, D_OUT], f32, kind="ExternalInput").ap()
    out = nc.dram_tensor("out", [C, D_OUT], f32, kind="ExternalOutput").ap()

    with tile.TileContext(nc) as tc:
        with (
            tc.tile_pool(name="xpool", bufs=1) as xpool,
            tc.tile_pool(name="wpool", bufs=1) as wpool,
            tc.tile_pool(name="cpool", bufs=1) as cpool,
            tc.tile_pool(name="opool", bufs=max(18, m_tiles + 1)) as opool,
            tc.tile_pool(name="pspool", bufs=8, space="PSUM") as pspool,
        ):
            x_sb = xpool.tile([P, m_tiles, KO, P], bf16)
            w_sb = wpool.tile([P, N_TILES, KO, N_TILE], bf16)
            b_sb = [
                cpool.tile([P, N_TILE], f32, name=f"b_sb{_n}")
                for _n in range(N_TILES)
            ]

            def x_tile_dma(m):
                nc.sync.dma_start(x_sb[:, m], xT[m])

            def w_slice_dma(n, ko0, ko1):
                nc.sync.dma_start(w_sb[:, n, ko0:ko1], wT[n, :, ko0:ko1])

            # DMA issue order == HBM service order (single serialized DMA
            # resource). x0 and x1 land first (the only PE-idle stretch),
            # then the n=0 W slab streams uninterrupted while a ko-outer
            # m={0,1} pair consumes each slice twice — 426 ns of PE work
            # per 364 ns slice, so the PE tracks the slab PE-bound.
            # 2-ko W slices for the n=0 slab: 728 ns transfer each, just
            # above the ~625 ns serialized HWDGE setup, so the slab
            # cadence is transfer-bound. Later slabs are fully prefetched
            # anyway and go as single 2 MB DMAs.
            nc.sync.dma_start(x_sb[:, 0, 0:4], xT[0][:, 0:4])
            nc.sync.dma_start(x_sb[:, 0, 4:KO], xT[0][:, 4:KO])
            w_slice_dma(0, 0, 1)
            if m_tiles > 1:
                x_tile_dma(1)
            w_slice_dma(0, 1, 2)
            for ko in range(2, KO, 2):
                w_slice_dma(0, ko, ko + 2)
            for m in range(2, min(4, m_tiles)):
                x_tile_dma(m)
            nc.sync.dma_start(b_sb[0][:], bb[:, 0:N_TILE])
            for m in range(4, m_tiles):
                x_tile_dma(m)
            for n in range(1, N_TILES):
                w_slice_dma(n, 0, KO)
                ns = slice(n * N_TILE, (n + 1) * N_TILE)
                nc.sync.dma_start(b_sb[n][:], bb[:, ns])

            def do_group(m, n, split_evict=False):
                ps = pspool.tile([P, N_TILE], f32, tag="ps")
                for ko in range(KO):
                    nc.tensor.matmul(
                        ps,
                        x_sb[:, m, ko],
                        w_sb[:, n, ko],
                        start=(ko == 0),
                        stop=(ko == KO - 1),
                    )
                ms = slice(m * P, (m + 1) * P)
                o_sb = opool.tile([P, N_TILE], f32, tag="o")
                if split_evict:
                    # last group: halve the post-matmul critical path
                    for h in range(2):
                        hs = slice(h * (N_TILE // 2), (h + 1) * (N_TILE // 2))
                        ns = slice(
                            n * N_TILE + h * (N_TILE // 2),
                            n * N_TILE + (h + 1) * (N_TILE // 2),
                        )
                        nc.vector.tensor_add(o_sb[:, hs], ps[:, hs], b_sb[n][:, hs])
                        nc.sync.dma_start(out[ms, ns], o_sb[:, hs])
                else:
                    ns = slice(n * N_TILE, (n + 1) * N_TILE)
                    nc.vector.tensor_add(o_sb[:], ps, b_sb[n][:])
                    nc.sync.dma_start(out[ms, ns], o_sb[:])

            def evict(ps, m, n):
                ms = slice(m * P, (m + 1) * P)
                ns = slice(n * N_TILE, (n + 1) * N_TILE)
                o_sb = opool.tile([P, N_TILE], f32, tag="o")
                nc.vector.tensor_add(o_sb[:], ps, b_sb[n][:])
                nc.sync.dma_start(out[ms, ns], o_sb[:])

            def start_pair(n):
                # ko-outer over m={0,1}: both matmuls of each ko need only
                # W slice ko, so each arriving slice feeds 2 matmuls.
                ps0 = pspool.tile([P, N_TILE], f32, tag="ps", name="ps0")
                ps1 = pspool.tile([P, N_TILE], f32, tag="ps", name="ps1")
                for ko in range(KO):
                    for m, ps in ((0, ps0), (1, ps1)):
                        nc.tensor.matmul(
                            ps,
                            x_sb[:, m, ko],
                            w_sb[:, n, ko],
                            start=(ko == 0),
                            stop=(ko == KO - 1),
                        )
                evict(ps0, 0, n)
                evict(ps1, 1, n)

            for _rep in range(repeat):
                for n in range(N_TILES):
                    m0 = 0
                    if _rep == 0 and n == 0 and m_tiles >= 2:
                        start_pair(n)
                        m0 = 2
                    for m in range(m0, m_tiles):
                        last = (
                            _rep == repeat - 1
                            and n == N_TILES - 1
                            and m == m_tiles - 1
                        )
                        do_group(m, n, split_evict=last)

    nc.compile()
    _nc_cache[key] = nc
    return nc


def shard_inputs(x, weight, bias, group_indices):
    """Host-side expert-parallel sharding. Returns (in_maps, perm,
    offsets, counts, C)."""
    n_rows = x.shape[0]
    gi = np.asarray(group_indices)
    perm = np.argsort(gi, kind="stable")
    counts = np.bincount(gi, minlength=NUM_GROUPS).astype(np.int64)
    offsets = np.zeros(NUM_GROUPS + 1, dtype=np.int64)
    np.cumsum(counts, out=offsets[1:])
    C = max(P, int(math.ceil(counts.max() / P)) * P)
    assert C // P <= 34, "x shard no longer fits SBUF-resident"

    x_sorted = x[perm] if not np.array_equal(perm, np.arange(n_rows)) else x
    m_tiles = C // P
    xbf = x_sorted.astype(ml_dtypes.bfloat16)
    wbf = np.asarray(weight).astype(ml_dtypes.bfloat16)
    in_maps = []
    for g in range(NUM_GROUPS):
        ng = int(counts[g])
        xg = np.zeros((C, D_IN), dtype=ml_dtypes.bfloat16)
        xg[:ng] = xbf[offsets[g] : offsets[g] + ng]
        # blocked layouts — see build_program
        xb = np.ascontiguousarray(
            xg.reshape(m_tiles, P, KO, P).transpose(0, 3, 2, 1)
        )
        wb = np.ascontiguousarray(
            wbf[g].T.reshape(KO, P, N_TILES, N_TILE).transpose(2, 1, 0, 3)
        )
        in_maps.append(
            {
                "xT": xb,
                "wT": wb,
                "bb": np.ascontiguousarray(
                    np.broadcast_to(bias[g].astype(np.float32), (P, D_OUT))
                ),
            }
        )
    return in_maps, perm, offsets, counts, C


def unshard_output(results, perm, offsets, counts, n_rows):
    out = np.empty((n_rows, D_OUT), dtype=np.float32)
    for g in range(NUM_GROUPS):
        ng = int(counts[g])
        out[perm[offsets[g] : offsets[g] + ng]] = results[g]["out"][:ng]
    return out


def kernel(x, weight, bias, group_indices):
    x = np.asarray(x, dtype=np.float32)
    weight = np.asarray(weight, dtype=np.float32)
    bias = np.asarray(bias, dtype=np.float32)
    group_indices = np.asarray(group_indices)
    assert x.shape[1] == D_IN and weight.shape == (NUM_GROUPS, D_OUT, D_IN)

    in_maps, perm, offsets, counts, C = shard_inputs(
        x, weight, bias, group_indices
    )
    nc = build_program(C)
    res = run_bass_kernel_spmd(nc, in_maps, core_ids=list(range(N_CORES)))
    return unshard_output(res.results, perm, offsets, counts, x.shape[0])
